# revision 27
# baseline (speedup 1.0000x reference)
"""Trainium2 Bass kernel for the DH-LIF node single-step forward.

Math: the mask is one-hot over the branch dim NB, so

    spike = ( (1-beta) * (x @ (W + M).T + b) >= 1 )
    M[h,i] = oma[h, idx[h,i]],   oma[h,k] = 0.5*(1 - sigmoid(tau_n[h,k]))

The host losslessly re-encodes the mask as the centered branch index
t' = idx - 1.5 (fp16, values +-0.5/+-1.5 exact).  The device rebuilds M via
the even/odd interpolation basis

    v = t'^2 (= 0.25 or 2.25, exact in fp16)
    M = (e0 + e1*v) + t'*(o0 + o1*v)

whose intermediates all stay below ~0.08, so per-step fp16 rounding is
harmless (45 flipped spikes of 239 allowed, deterministic inputs).  M is
transposed on TensorE (fp16 identity, 1 cyc/row), the +W^T rides the
PSUM->SBUF evacuation (W^T ships pre-transposed fp16), and one fp16 x fp8
matmul per 128-chunk accumulates into f32 PSUM (x ships as fp8; spikes are
0/1, exact).  Threshold compares against 1/(1-beta) - b; the 0/1 result is
written back as fp8.

Contraction index layout: i = p*32 + c (p = SBUF partition, c = k-chunk), a
pure host reshape that makes the x / W^T DMAs fully contiguous per
partition (128 descriptors of 4 KiB per quarter-tensor transfer).

Engine split per supertile: Act {v = Square(t'), q2 = o1*v + o0 via
Identity scale/bias, W^T DMAs}, Pool {q1 = e1*v + e0, x DMAs},
DVE {q3 = q2*t', P = q1 + q3, wc = psum + W^T, spike threshold},
PE {transposes + matmuls + clock-warming dummies}, SP {idx/param/out DMAs}.

Sharding: hidden dim split across 8 cores (h_loc = 256); x replicated.
"""

import numpy as np
import ml_dtypes

B, I, H, NB = 512, 4096, 2048, 4
NCORES = 8
H_LOC = H // NCORES          # 256
N_HT = H_LOC // 128          # 2 partition tiles of hidden per core
N_CHUNK = I // 128           # 32 matmul k-chunks

# (ht, base_chunk, n_chunks): small supers at the head (fast pipeline fill)
# and tail (short drain), 8-chunk supers in the middle.
SUPERS = [
    (0, 0, 4), (0, 4, 4), (0, 8, 8), (1, 0, 8), (0, 16, 8),
    (1, 8, 8), (1, 16, 8), (0, 24, 8), (1, 24, 4), (1, 28, 4),
]
# idx DMA pieces: (ht, col_start, col_end), ordered by consumption
IDX_PIECES = [
    (0, 0, 1024), (0, 1024, 2048), (1, 0, 2048),
    (0, 2048, 4096), (1, 2048, 3072), (1, 3072, 4096),
]
# x DMA pieces: chunk ranges; the last piece feeds only the tail matmuls
X_PIECES = [(0, 8), (8, 16), (16, 24), (24, 28), (28, 32)]
NWARM = 38

TRACE = False
LAST_RESULTS = None
_CACHED = {}


def _build_bass(reps=1):
    import concourse.bacc as bacc
    import concourse.mybir as mybir
    from concourse.tile import TileContext
    from concourse.masks import make_identity

    f32 = mybir.dt.float32
    fp16 = mybir.dt.float16
    fp8 = mybir.dt.float8e4
    AF = mybir.ActivationFunctionType
    ALU = mybir.AluOpType

    nc = bacc.Bacc("TRN2", target_bir_lowering=False, debug=False)

    x_in = nc.dram_tensor("x", [128, N_CHUNK, B], fp8, kind="ExternalInput")
    wT_in = nc.dram_tensor("wT", [128, N_CHUNK, H_LOC], fp16, kind="ExternalInput")
    idx_in = nc.dram_tensor("idx", [H_LOC, I], fp16, kind="ExternalInput")
    par_in = nc.dram_tensor("par", [128, 2 * (NB + 2)], f32, kind="ExternalInput")
    out = nc.dram_tensor("out", [H_LOC, B], fp8, kind="ExternalOutput")

    with TileContext(nc) as tc:
        with (
            tc.tile_pool(name="const", bufs=1) as const_pool,
            tc.tile_pool(name="xp", bufs=3) as x_pool,
            tc.tile_pool(name="wt", bufs=4) as wt_pool,
            tc.tile_pool(name="ix", bufs=len(IDX_PIECES)) as idx_pool,
            tc.tile_pool(name="pl", bufs=3) as plane_pool,
            tc.tile_pool(name="wc", bufs=3) as wc_pool,
            tc.tile_pool(name="res", bufs=2) as res_pool,
            tc.tile_pool(name="pt", bufs=2, space="PSUM") as psum_t_pool,
            tc.tile_pool(name="po", bufs=2, space="PSUM") as psum_o_pool,
            tc.tile_pool(name="pw", bufs=1, space="PSUM") as psum_w_pool,
        ):
            ident = const_pool.tile([128, 128], fp16)
            make_identity(nc, ident)

            # Clock warmup: keep the PE continuously busy from t=0 so it is
            # at 2.4 GHz (needs >3us back-to-back) when real work arrives.
            warm = psum_w_pool.tile([128, 128], f32, name="warm")

            def pad(n):
                for _ in range(n):
                    nc.tensor.matmul(warm[:], ident[:], ident[:],
                                     start=True, stop=True,
                                     skip_group_check=True)

            pad(NWARM)
            for rep in range(reps):
                _emit_rep(nc, tc, rep, ident, pad,
                          const_pool, x_pool, wt_pool, idx_pool, plane_pool,
                          wc_pool, res_pool, psum_t_pool, psum_o_pool,
                          x_in, wT_in, idx_in, par_in, out,
                          f32, fp16, fp8, AF, ALU)

    nc.compile()
    return nc


def _emit_rep(nc, tc, rep, ident, pad,
              const_pool, x_pool, wt_pool, idx_pool, plane_pool,
              wc_pool, res_pool, psum_t_pool, psum_o_pool,
              x_in, wT_in, idx_in, par_in, out,
              f32, fp16, fp8, AF, ALU):
    R = f"r{rep}_"
    n_sup = len(SUPERS)

    # ---- DMA issue, spread over three queues ----
    # Each DMA instruction holds its queue until the transfer completes, so
    # the head-of-pipeline pieces go on separate queues: idx piece 0 leads
    # the SP queue, the (single, packed) param tile rides Pool's SWDGE.
    par_all = const_pool.tile([128, 2 * (NB + 2)], f32, tag=f"{R}par",
                              name=f"{R}par")
    nc.gpsimd.dma_start(par_all[:], par_in[:, :])

    def par_col(ht, c0, c1):
        off = ht * (NB + 2)
        return par_all[:, off + c0:off + c1]

    idx_sb = {}

    def idx_piece_tile(pi):
        ht, c0, c1 = IDX_PIECES[pi]
        t = idx_pool.tile([128, c1 - c0], fp16, tag=f"ix{c1-c0}",
                          name=f"{R}ix{pi}")
        nc.sync.dma_start(t[:], idx_in[ht * 128:(ht + 1) * 128, c0:c1])
        idx_sb[pi] = t

    for pi in range(len(IDX_PIECES)):
        idx_piece_tile(pi)

    def idx_slice(ht, c0, c1):
        """SBUF view of centered-idx columns [c0*128, c1*128) of h-tile ht."""
        lo, hi = c0 * 128, c1 * 128
        for pi, (h, p0, p1) in enumerate(IDX_PIECES):
            if h == ht and p0 <= lo and hi <= p1:
                return idx_sb[pi][:, lo - p0:hi - p0]
        raise AssertionError((ht, c0, c1))

    # Act queue: dummy sigmoid first — forces the single activation-table
    # load (Sigmoid/Square/Identity share a set) off the critical path.
    # The sigmoids also gate Act's first bulk DMA so the idx pieces win the
    # DMA-engine queue.
    scr = const_pool.tile([128, 1], f32, tag=f"{R}scr", name=f"{R}scr")
    nc.vector.memset(scr[:], 0.0)
    scr2 = const_pool.tile([128, 1], f32, tag=f"{R}scr2", name=f"{R}scr2")
    nc.scalar.activation(scr2[:], scr[:], AF.Sigmoid)

    sig_t = []
    for ht in range(N_HT):
        sig_n = const_pool.tile([128, NB], f32, tag=f"{R}sn{ht}", name=f"{R}sn{ht}")
        nc.scalar.activation(sig_n[:], par_col(ht, 0, NB), AF.Sigmoid)
        beta = const_pool.tile([128, 1], f32, tag=f"{R}be{ht}", name=f"{R}be{ht}")
        nc.scalar.activation(beta[:], par_col(ht, NB, NB + 1), AF.Sigmoid)
        sig_t.append((sig_n, beta))

    wt_t = []

    def dma_wt(g):
        t = wt_pool.tile([128, 8, H_LOC], fp16, tag="wt8", name=f"{R}wt{g}")
        nc.scalar.dma_start(t[:], wT_in[:, g * 8:(g + 1) * 8, :])
        wt_t.append(t)

    dma_wt(0)

    # x pieces ride the Act HWDGE queue (the tail pieces go via SP once its
    # idx pieces are through), interleaved with the v stages.
    x_t = {}

    def dma_x(g, eng=None):
        c0, c1 = X_PIECES[g]
        t = x_pool.tile([128, c1 - c0, B], fp8, tag=f"x{c1-c0}", name=f"{R}x{g}")
        (eng or nc.scalar).dma_start(t[:], x_in[:, c0:c1, :])
        x_t[g] = t

    def x_slice(c):
        for g, (c0, c1) in enumerate(X_PIECES):
            if c0 <= c < c1:
                return x_t[g][:, c - c0, :]
        raise AssertionError(c)

    # ---- coefficients (f32, tiny): even/odd basis over v = t'^2 ----
    #   E(v) = e0 + e1 v   through ((o1+o2)/2 @v=.25, (o0+o3)/2 @v=2.25)
    #   O(v) = o0_ + o1_ v through (o2-o1 @v=.25, (o3-o0)/3 @v=2.25)
    coef = []   # (e0, e1, oo0, oo1) APs per ht
    thr_t = []
    for ht in range(N_HT):
        sig_n, beta = sig_t[ht]
        oma = const_pool.tile([128, NB], f32, tag=f"{R}oma{ht}", name=f"{R}oma{ht}")
        # 0.5 * (1 - sigmoid(tau_n)) — includes the 0.5 dendritic scale
        nc.vector.tensor_scalar(oma[:], sig_n[:], -0.5, 0.5, op0=ALU.mult, op1=ALU.add)

        sc = const_pool.tile([128, 8], f32, tag=f"{R}sc{ht}", name=f"{R}sc{ht}")
        s_in, s_out = sc[:, 0:1], sc[:, 1:2]
        e0, e1, oo0, oo1 = sc[:, 2:3], sc[:, 3:4], sc[:, 4:5], sc[:, 5:6]
        O_in, O_out = sc[:, 6:7], sc[:, 7:8]
        nc.vector.tensor_tensor(s_in, oma[:, 1:2], oma[:, 2:3], ALU.add)
        nc.vector.tensor_tensor(s_out, oma[:, 0:1], oma[:, 3:4], ALU.add)
        # e1 = (s_out - s_in)/4 ; e0 = s_in/2 - e1/4
        nc.vector.tensor_tensor(e1, s_out, s_in, ALU.subtract)
        nc.vector.tensor_scalar(e1, e1, 0.25, None, op0=ALU.mult)
        nc.vector.tensor_scalar(t_ := sc[:, 0:1], s_in, 0.5, None, op0=ALU.mult)
        nc.vector.tensor_scalar(s_out, e1, -0.25, None, op0=ALU.mult)
        nc.vector.tensor_tensor(e0, t_, s_out, ALU.add)
        nc.vector.tensor_tensor(O_in, oma[:, 2:3], oma[:, 1:2], ALU.subtract)
        nc.vector.tensor_tensor(O_out, oma[:, 3:4], oma[:, 0:1], ALU.subtract)
        nc.vector.tensor_scalar(O_out, O_out, 1.0 / 3.0, None, op0=ALU.mult)
        # oo1 = (O_out - O_in)/2 ; oo0 = O_in - oo1/4
        nc.vector.tensor_tensor(oo1, O_out, O_in, ALU.subtract)
        nc.vector.tensor_scalar(oo1, oo1, 0.5, None, op0=ALU.mult)
        nc.vector.tensor_scalar(O_out, oo1, -0.25, None, op0=ALU.mult)
        nc.vector.tensor_tensor(oo0, O_in, O_out, ALU.add)
        coef.append((e0, e1, oo0, oo1))

        omb = const_pool.tile([128, 1], f32, tag=f"{R}ob{ht}", name=f"{R}ob{ht}")
        nc.vector.tensor_scalar(omb[:], beta[:], -1.0, 1.0, op0=ALU.mult, op1=ALU.add)
        rb = const_pool.tile([128, 1], f32, tag=f"{R}rb{ht}", name=f"{R}rb{ht}")
        nc.vector.reciprocal(rb[:], omb[:])
        thr = const_pool.tile([128, 1], f32, tag=f"{R}th{ht}", name=f"{R}th{ht}")
        nc.vector.tensor_tensor(thr[:], rb[:], par_col(ht, NB + 1, NB + 2),
                                ALU.subtract)
        thr_t.append(thr)

    # ---- main pipeline ----
    psum_out = [None] * N_HT
    for ht in range(N_HT):
        psum_out[ht] = psum_o_pool.tile([128, B], f32, tag="po", name=f"{R}po{ht}")

    v_t = [None] * n_sup
    q1_t = [None] * n_sup
    q2_t = [None] * n_sup
    q3_t = [None] * n_sup
    pt_t = [None] * n_sup
    wc_t = [None] * n_sup

    def tp(pool, k, nm):
        ht, base, njc = SUPERS[k]
        return pool.tile([128, njc * 128], fp16, tag=f"{nm}{njc}",
                         name=f"{R}{nm}{k}")

    def sV(k):  # Act: v = (t')^2
        ht, base, njc = SUPERS[k]
        v = tp(plane_pool, k, "v")
        v_t[k] = v
        nc.scalar.activation(v[:], idx_slice(ht, base, base + njc), AF.Square)

    def sQ2(k):  # DVE: q2 = o1*v + o0  (tensor_scalar runs at 4x on fp16)
        ht, base, njc = SUPERS[k]
        _, _, oo0, oo1 = coef[ht]
        q2 = tp(plane_pool, k, "q2")
        q2_t[k] = q2
        nc.vector.tensor_scalar(q2[:], v_t[k][:], oo1, oo0, op0=ALU.mult, op1=ALU.add)

    def sQ1(k):  # Pool: q1 = e1*v + e0
        ht, base, njc = SUPERS[k]
        e0, e1, _, _ = coef[ht]
        q1 = tp(plane_pool, k, "q1")
        q1_t[k] = q1
        nc.gpsimd.tensor_scalar(q1[:], v_t[k][:], e1, e0, op0=ALU.mult, op1=ALU.add)

    def sQ3(k):  # DVE: q3 = q2 * t'
        ht, base, njc = SUPERS[k]
        q3 = tp(plane_pool, k, "q3")
        q3_t[k] = q3
        nc.vector.tensor_tensor(q3[:], q2_t[k][:], idx_slice(ht, base, base + njc),
                                ALU.mult)
        v_t[k] = None

    def sP(k):  # DVE: P = q1 + q3  (final M plane, in-place on q3)
        nc.vector.tensor_tensor(q3_t[k][:], q1_t[k][:], q3_t[k][:], ALU.add)
        q1_t[k] = None
        q2_t[k] = None

    def sF(k):  # PE: transposes into fp16 PSUM
        ht, base, njc = SUPERS[k]
        pt = psum_t_pool.tile([128, njc, 128], fp16, tag=f"pt{njc}",
                              name=f"{R}pt{k}")
        pt_t[k] = pt
        P = q3_t[k]
        for j in range(njc):
            nc.tensor.transpose(pt[:, j, :], P[:, j * 128:(j + 1) * 128], ident[:])

    def sG(k):  # DVE: wc = pt + W^T
        ht, base, njc = SUPERS[k]
        wc = wc_pool.tile([128, njc, 128], fp16, tag=f"wc{njc}", name=f"{R}wc{k}")
        wc_t[k] = wc
        hsl = slice(ht * 128, (ht + 1) * 128)
        g, off = divmod(base, 8)
        nc.vector.tensor_tensor(wc[:], pt_t[k][:],
                                wt_t[g][:, off:off + njc, hsl], ALU.add)
        q3_t[k] = None
        pt_t[k] = None

    def sH(k):  # PE: matmuls, rhs = x fp8
        ht, base, njc = SUPERS[k]
        po = psum_out[ht]
        wc = wc_t[k]
        for j in range(njc):
            c = base + j
            nc.tensor.matmul(
                po[:], wc[:, j, :], x_slice(c),
                start=(c == 0), stop=(c == N_CHUNK - 1),
                skip_group_check=True,
            )
        wc_t[k] = None

    # remaining bulk DMAs are interleaved into the queues as the pipeline
    # advances, roughly in consumption order ("sx" = x piece via SP)
    dma_at = {0: [("x", 0)], 1: [("x", 1)], 2: [("wt", 1)],
              3: [("x", 2), ("wt", 2)], 4: [("wt", 3)],
              5: [("sx", 3)], 6: [("sx", 4)]}
    # PE padding after early stages: absorbs dependency gaps so the tensor
    # engine never idles (an idle gap resets its clock to 1.2 GHz for 3us)
    pe_pad = {0: 8, 1: 6, 2: 4, 3: 2}

    for k in range(n_sup + 3):
        if k < n_sup:
            sV(k)
        # DVE step: every op's inputs were produced >= 1 step ago, and the
        # matmul-feeding evacuation (sG) leads so it is never head-of-line
        # blocked behind a stalled plane op.
        if 0 <= k - 2 < n_sup:
            sF(k - 2)
            sG(k - 2)
            pad(pe_pad.get(k - 2, 0))
        if 0 <= k - 1 < n_sup:
            sQ2(k - 1)
            sQ3(k - 1)
            sP(k - 1)
        if k < n_sup:
            sQ1(k)
        for kind, g in dma_at.get(k, []):
            if kind == "x":
                dma_x(g)
            elif kind == "sx":
                dma_x(g, eng=nc.sync)
            else:
                dma_wt(g)
        if 0 <= k - 3 < n_sup:
            sH(k - 3)

    for ht in range(N_HT):
        res = res_pool.tile([128, B], fp8, tag="res", name=f"{R}res{ht}")
        nc.vector.tensor_scalar(
            res[:], psum_out[ht][:], thr_t[ht][:], None, op0=ALU.is_ge
        )
        # separate queues so the two output transfers overlap
        eng = nc.sync if ht == 0 else nc.scalar
        eng.dma_start(out[ht * 128:(ht + 1) * 128, :], res[:])


def _get_nc(reps=1):
    key = f"nc{reps}"
    if key not in _CACHED:
        _CACHED[key] = _build_bass(reps)
    return _CACHED[key]


def kernel(**inputs):
    global LAST_RESULTS
    from concourse.bass_utils import run_bass_kernel_spmd

    x = np.asarray(inputs["x"], dtype=np.float32)
    W = np.asarray(inputs["W"], dtype=np.float32)
    b = np.asarray(inputs["b"], dtype=np.float32)
    tau_m = np.asarray(inputs["tau_m"], dtype=np.float32)
    tau_n = np.asarray(inputs["tau_n"], dtype=np.float32)
    mask = np.asarray(inputs["mask"], dtype=np.float32)

    fp16 = np.float16
    fp8 = ml_dtypes.float8_e4m3
    # contraction layout i = p*32 + c (pure reshape of the transposed x / W)
    xr = np.ascontiguousarray(x.T).reshape(128, N_CHUNK, B).astype(fp8)
    # centered branch index t' = idx - 1.5, columns permuted to (c*128 + p)
    idx = (mask[:, :, 1] + 2.0 * mask[:, :, 2] + 3.0 * mask[:, :, 3]) - 1.5
    idx = np.ascontiguousarray(
        idx.reshape(H, 128, N_CHUNK).swapaxes(1, 2).reshape(H, I)
    ).astype(fp16)
    W16 = W.astype(fp16)

    nc = _get_nc()
    in_maps = []
    for c in range(NCORES):
        hs = slice(c * H_LOC, (c + 1) * H_LOC)
        par6 = np.concatenate(
            [tau_n[hs], tau_m[hs, None], b[hs, None]], axis=1
        ).astype(np.float32)                       # [256, 6]
        par = np.concatenate([par6[0:128], par6[128:256]], axis=1)  # [128, 12]
        in_maps.append({
            "x": xr,
            "wT": np.ascontiguousarray(W16[hs].T.reshape(128, N_CHUNK, H_LOC)),
            "idx": np.ascontiguousarray(idx[hs]),
            "par": np.ascontiguousarray(par),
        })

    try:
        res = run_bass_kernel_spmd(
            nc, in_maps, core_ids=list(range(NCORES)), trace=TRACE,
        )
    except Exception:
        if not TRACE:
            raise
        # tracing needs the NTFF profiling hook, which not every
        # environment provides — rerun without it
        res = run_bass_kernel_spmd(
            nc, in_maps, core_ids=list(range(NCORES)), trace=False,
        )
    LAST_RESULTS = res
    outT = np.concatenate(
        [np.asarray(r["out"], dtype=np.float32) for r in res.results], axis=0
    )                                                                 # [H, B]
    return np.ascontiguousarray(outT.T)                               # [B, H]


# revision 28
# speedup vs baseline: 1.0009x; 1.0009x over previous
"""Trainium2 Bass kernel for the DH-LIF node single-step forward.

Math: the mask is one-hot over the branch dim NB, so

    spike = ( (1-beta) * (x @ (W + M).T + b) >= 1 )
    M[h,i] = oma[h, idx[h,i]],   oma[h,k] = 0.5*(1 - sigmoid(tau_n[h,k]))

The host losslessly re-encodes the mask as the centered branch index
t' = idx - 1.5 (fp16, values +-0.5/+-1.5 exact).  The device rebuilds M via
the even/odd interpolation basis

    v = t'^2 (= 0.25 or 2.25, exact in fp16)
    M = (e0 + e1*v) + t'*(o0 + o1*v)

whose intermediates all stay below ~0.08, so per-step fp16 rounding is
harmless (45 flipped spikes of 239 allowed, deterministic inputs).  M is
transposed on TensorE (fp16 identity, 1 cyc/row), the +W^T rides the
PSUM->SBUF evacuation (W^T ships pre-transposed fp16), and one fp16 x fp8
matmul per 128-chunk accumulates into f32 PSUM (x ships as fp8; spikes are
0/1, exact).  Threshold compares against 1/(1-beta) - b; the 0/1 result is
written back as fp8.

Contraction index layout: i = p*32 + c (p = SBUF partition, c = k-chunk), a
pure host reshape that makes the x / W^T DMAs fully contiguous per
partition (128 descriptors of 4 KiB per quarter-tensor transfer).

Engine split per supertile: Act {v = Square(t'), q2 = o1*v + o0 via
Identity scale/bias, W^T DMAs}, Pool {q1 = e1*v + e0, x DMAs},
DVE {q3 = q2*t', P = q1 + q3, wc = psum + W^T, spike threshold},
PE {transposes + matmuls + clock-warming dummies}, SP {idx/param/out DMAs}.

Sharding: hidden dim split across 8 cores (h_loc = 256); x replicated.
"""

import numpy as np
import ml_dtypes

B, I, H, NB = 512, 4096, 2048, 4
NCORES = 8
H_LOC = H // NCORES          # 256
N_HT = H_LOC // 128          # 2 partition tiles of hidden per core
N_CHUNK = I // 128           # 32 matmul k-chunks

# (ht, base_chunk, n_chunks): small supers at the head (fast pipeline fill)
# and tail (short drain), 8-chunk supers in the middle.
SUPERS = [
    (0, 0, 4), (0, 4, 4), (0, 8, 8), (1, 0, 8), (0, 16, 8),
    (1, 8, 8), (1, 16, 8), (0, 24, 8), (1, 24, 4), (1, 28, 4),
]
# idx DMA pieces: (ht, col_start, col_end), ordered by consumption
IDX_PIECES = [
    (0, 0, 1024), (0, 1024, 2048), (1, 0, 2048),
    (0, 2048, 4096), (1, 2048, 3072), (1, 3072, 4096),
]
# x DMA pieces: chunk ranges; the last piece feeds only the tail matmuls
X_PIECES = [(0, 8), (8, 16), (16, 24), (24, 28), (28, 32)]
NWARM = 38

TRACE = False
LAST_RESULTS = None
_CACHED = {}


def _build_bass(reps=1):
    import concourse.bacc as bacc
    import concourse.mybir as mybir
    from concourse.tile import TileContext
    from concourse.masks import make_identity

    f32 = mybir.dt.float32
    fp16 = mybir.dt.float16
    fp8 = mybir.dt.float8e4
    AF = mybir.ActivationFunctionType
    ALU = mybir.AluOpType

    nc = bacc.Bacc("TRN2", target_bir_lowering=False, debug=False)

    x_in = nc.dram_tensor("x", [128, N_CHUNK, B], fp8, kind="ExternalInput")
    wT_in = nc.dram_tensor("wT", [128, N_CHUNK, H_LOC], fp16, kind="ExternalInput")
    idx_in = nc.dram_tensor("idx", [H_LOC, I], fp16, kind="ExternalInput")
    par_in = nc.dram_tensor("par", [128, 2 * (NB + 2)], f32, kind="ExternalInput")
    out = nc.dram_tensor("out", [H_LOC, B], fp8, kind="ExternalOutput")

    with TileContext(nc) as tc:
        with (
            tc.tile_pool(name="const", bufs=1) as const_pool,
            tc.tile_pool(name="xp", bufs=3) as x_pool,
            tc.tile_pool(name="wt", bufs=4) as wt_pool,
            tc.tile_pool(name="ix", bufs=len(IDX_PIECES)) as idx_pool,
            tc.tile_pool(name="pl", bufs=3) as plane_pool,
            tc.tile_pool(name="wc", bufs=3) as wc_pool,
            tc.tile_pool(name="res", bufs=2) as res_pool,
            tc.tile_pool(name="pt", bufs=2, space="PSUM") as psum_t_pool,
            tc.tile_pool(name="po", bufs=2, space="PSUM") as psum_o_pool,
            tc.tile_pool(name="pw", bufs=1, space="PSUM") as psum_w_pool,
        ):
            ident = const_pool.tile([128, 128], fp16)
            make_identity(nc, ident)

            # Clock warmup: keep the PE continuously busy from t=0 so it is
            # at 2.4 GHz (needs >3us back-to-back) when real work arrives.
            warm = psum_w_pool.tile([128, 128], f32, name="warm")

            def pad(n):
                for _ in range(n):
                    nc.tensor.matmul(warm[:], ident[:], ident[:],
                                     start=True, stop=True,
                                     skip_group_check=True)

            pad(NWARM)
            for rep in range(reps):
                _emit_rep(nc, tc, rep, ident, pad,
                          const_pool, x_pool, wt_pool, idx_pool, plane_pool,
                          wc_pool, res_pool, psum_t_pool, psum_o_pool,
                          x_in, wT_in, idx_in, par_in, out,
                          f32, fp16, fp8, AF, ALU)

    nc.compile()
    return nc


def _emit_rep(nc, tc, rep, ident, pad,
              const_pool, x_pool, wt_pool, idx_pool, plane_pool,
              wc_pool, res_pool, psum_t_pool, psum_o_pool,
              x_in, wT_in, idx_in, par_in, out,
              f32, fp16, fp8, AF, ALU):
    R = f"r{rep}_"
    n_sup = len(SUPERS)

    # ---- DMA issue, spread over three queues ----
    # Each DMA instruction holds its queue until the transfer completes, so
    # the head-of-pipeline pieces go on separate queues: idx piece 0 leads
    # the SP queue, the (single, packed) param tile rides Pool's SWDGE.
    par_all = const_pool.tile([128, 2 * (NB + 2)], f32, tag=f"{R}par",
                              name=f"{R}par")
    nc.gpsimd.dma_start(par_all[:], par_in[:, :])

    def par_col(ht, c0, c1):
        off = ht * (NB + 2)
        return par_all[:, off + c0:off + c1]

    idx_sb = {}

    def idx_piece_tile(pi):
        ht, c0, c1 = IDX_PIECES[pi]
        t = idx_pool.tile([128, c1 - c0], fp16, tag=f"ix{c1-c0}",
                          name=f"{R}ix{pi}")
        nc.sync.dma_start(t[:], idx_in[ht * 128:(ht + 1) * 128, c0:c1])
        idx_sb[pi] = t

    for pi in range(len(IDX_PIECES)):
        idx_piece_tile(pi)

    def idx_slice(ht, c0, c1):
        """SBUF view of centered-idx columns [c0*128, c1*128) of h-tile ht."""
        lo, hi = c0 * 128, c1 * 128
        for pi, (h, p0, p1) in enumerate(IDX_PIECES):
            if h == ht and p0 <= lo and hi <= p1:
                return idx_sb[pi][:, lo - p0:hi - p0]
        raise AssertionError((ht, c0, c1))

    # Act queue: dummy sigmoid first — forces the single activation-table
    # load (Sigmoid/Square/Identity share a set) off the critical path.
    # The sigmoids also gate Act's first bulk DMA so the idx pieces win the
    # DMA-engine queue.
    scr = const_pool.tile([128, 1], f32, tag=f"{R}scr", name=f"{R}scr")
    nc.vector.memset(scr[:], 0.0)
    scr2 = const_pool.tile([128, 1], f32, tag=f"{R}scr2", name=f"{R}scr2")
    nc.scalar.activation(scr2[:], scr[:], AF.Sigmoid)

    sig_t = []
    for ht in range(N_HT):
        sig_n = const_pool.tile([128, NB], f32, tag=f"{R}sn{ht}", name=f"{R}sn{ht}")
        nc.scalar.activation(sig_n[:], par_col(ht, 0, NB), AF.Sigmoid)
        beta = const_pool.tile([128, 1], f32, tag=f"{R}be{ht}", name=f"{R}be{ht}")
        nc.scalar.activation(beta[:], par_col(ht, NB, NB + 1), AF.Sigmoid)
        sig_t.append((sig_n, beta))

    wt_t = []

    def dma_wt(g):
        t = wt_pool.tile([128, 8, H_LOC], fp16, tag="wt8", name=f"{R}wt{g}")
        nc.scalar.dma_start(t[:], wT_in[:, g * 8:(g + 1) * 8, :])
        wt_t.append(t)

    dma_wt(0)

    # x pieces ride the Act HWDGE queue (the tail pieces go via SP once its
    # idx pieces are through), interleaved with the v stages.
    x_t = {}

    def dma_x(g, eng=None):
        c0, c1 = X_PIECES[g]
        t = x_pool.tile([128, c1 - c0, B], fp8, tag=f"x{c1-c0}", name=f"{R}x{g}")
        (eng or nc.scalar).dma_start(t[:], x_in[:, c0:c1, :])
        x_t[g] = t

    def x_slice(c):
        for g, (c0, c1) in enumerate(X_PIECES):
            if c0 <= c < c1:
                return x_t[g][:, c - c0, :]
        raise AssertionError(c)

    # ---- coefficients (f32, tiny): even/odd basis over v = t'^2 ----
    #   E(v) = e0 + e1 v   through ((o1+o2)/2 @v=.25, (o0+o3)/2 @v=2.25)
    #   O(v) = o0_ + o1_ v through (o2-o1 @v=.25, (o3-o0)/3 @v=2.25)
    coef = []   # (e0, e1, oo0, oo1) APs per ht
    thr_t = []
    for ht in range(N_HT):
        sig_n, beta = sig_t[ht]
        oma = const_pool.tile([128, NB], f32, tag=f"{R}oma{ht}", name=f"{R}oma{ht}")
        # 0.5 * (1 - sigmoid(tau_n)) — includes the 0.5 dendritic scale
        nc.vector.tensor_scalar(oma[:], sig_n[:], -0.5, 0.5, op0=ALU.mult, op1=ALU.add)

        sc = const_pool.tile([128, 8], f32, tag=f"{R}sc{ht}", name=f"{R}sc{ht}")
        s_in, s_out = sc[:, 0:1], sc[:, 1:2]
        e0, e1, oo0, oo1 = sc[:, 2:3], sc[:, 3:4], sc[:, 4:5], sc[:, 5:6]
        O_in, O_out = sc[:, 6:7], sc[:, 7:8]
        nc.vector.tensor_tensor(s_in, oma[:, 1:2], oma[:, 2:3], ALU.add)
        nc.vector.tensor_tensor(s_out, oma[:, 0:1], oma[:, 3:4], ALU.add)
        # e1 = (s_out - s_in)/4 ; e0 = s_in/2 - e1/4
        nc.vector.tensor_tensor(e1, s_out, s_in, ALU.subtract)
        nc.vector.tensor_scalar(e1, e1, 0.25, None, op0=ALU.mult)
        nc.vector.tensor_scalar(t_ := sc[:, 0:1], s_in, 0.5, None, op0=ALU.mult)
        nc.vector.tensor_scalar(s_out, e1, -0.25, None, op0=ALU.mult)
        nc.vector.tensor_tensor(e0, t_, s_out, ALU.add)
        nc.vector.tensor_tensor(O_in, oma[:, 2:3], oma[:, 1:2], ALU.subtract)
        nc.vector.tensor_tensor(O_out, oma[:, 3:4], oma[:, 0:1], ALU.subtract)
        nc.vector.tensor_scalar(O_out, O_out, 1.0 / 3.0, None, op0=ALU.mult)
        # oo1 = (O_out - O_in)/2 ; oo0 = O_in - oo1/4
        nc.vector.tensor_tensor(oo1, O_out, O_in, ALU.subtract)
        nc.vector.tensor_scalar(oo1, oo1, 0.5, None, op0=ALU.mult)
        nc.vector.tensor_scalar(O_out, oo1, -0.25, None, op0=ALU.mult)
        nc.vector.tensor_tensor(oo0, O_in, O_out, ALU.add)
        coef.append((e0, e1, oo0, oo1))

        omb = const_pool.tile([128, 1], f32, tag=f"{R}ob{ht}", name=f"{R}ob{ht}")
        nc.vector.tensor_scalar(omb[:], beta[:], -1.0, 1.0, op0=ALU.mult, op1=ALU.add)
        rb = const_pool.tile([128, 1], f32, tag=f"{R}rb{ht}", name=f"{R}rb{ht}")
        nc.vector.reciprocal(rb[:], omb[:])
        thr = const_pool.tile([128, 1], f32, tag=f"{R}th{ht}", name=f"{R}th{ht}")
        nc.vector.tensor_tensor(thr[:], rb[:], par_col(ht, NB + 1, NB + 2),
                                ALU.subtract)
        thr_t.append(thr)

    # ---- main pipeline ----
    psum_out = [None] * N_HT
    for ht in range(N_HT):
        psum_out[ht] = psum_o_pool.tile([128, B], f32, tag="po", name=f"{R}po{ht}")

    v_t = [None] * n_sup
    q1_t = [None] * n_sup
    q2_t = [None] * n_sup
    q3_t = [None] * n_sup
    pt_t = [None] * n_sup
    wc_t = [None] * n_sup

    def tp(pool, k, nm):
        ht, base, njc = SUPERS[k]
        return pool.tile([128, njc * 128], fp16, tag=f"{nm}{njc}",
                         name=f"{R}{nm}{k}")

    def sV(k):  # Act: v = (t')^2
        ht, base, njc = SUPERS[k]
        v = tp(plane_pool, k, "v")
        v_t[k] = v
        nc.scalar.activation(v[:], idx_slice(ht, base, base + njc), AF.Square)

    def sQ2(k):  # DVE: q2 = o1*v + o0  (tensor_scalar runs at 4x on fp16)
        ht, base, njc = SUPERS[k]
        _, _, oo0, oo1 = coef[ht]
        q2 = tp(plane_pool, k, "q2")
        q2_t[k] = q2
        nc.vector.tensor_scalar(q2[:], v_t[k][:], oo1, oo0, op0=ALU.mult, op1=ALU.add)

    def sQ1(k):  # Pool: q1 = e1*v + e0
        ht, base, njc = SUPERS[k]
        e0, e1, _, _ = coef[ht]
        q1 = tp(plane_pool, k, "q1")
        q1_t[k] = q1
        nc.gpsimd.tensor_scalar(q1[:], v_t[k][:], e1, e0, op0=ALU.mult, op1=ALU.add)

    def sQ3(k):  # DVE: q3 = q2 * t'
        ht, base, njc = SUPERS[k]
        q3 = tp(plane_pool, k, "q3")
        q3_t[k] = q3
        nc.vector.tensor_tensor(q3[:], q2_t[k][:], idx_slice(ht, base, base + njc),
                                ALU.mult)
        v_t[k] = None

    def sP(k):  # DVE: P = q1 + q3  (final M plane, in-place on q3)
        nc.vector.tensor_tensor(q3_t[k][:], q1_t[k][:], q3_t[k][:], ALU.add)
        q1_t[k] = None
        q2_t[k] = None

    def sF(k):  # PE: transposes into fp16 PSUM
        ht, base, njc = SUPERS[k]
        pt = psum_t_pool.tile([128, njc, 128], fp16, tag=f"pt{njc}",
                              name=f"{R}pt{k}")
        pt_t[k] = pt
        P = q3_t[k]
        for j in range(njc):
            nc.tensor.transpose(pt[:, j, :], P[:, j * 128:(j + 1) * 128], ident[:])

    def sG(k):  # DVE: wc = pt + W^T
        ht, base, njc = SUPERS[k]
        wc = wc_pool.tile([128, njc, 128], fp16, tag=f"wc{njc}", name=f"{R}wc{k}")
        wc_t[k] = wc
        hsl = slice(ht * 128, (ht + 1) * 128)
        g, off = divmod(base, 8)
        nc.vector.tensor_tensor(wc[:], pt_t[k][:],
                                wt_t[g][:, off:off + njc, hsl], ALU.add)
        q3_t[k] = None
        pt_t[k] = None

    def sH(k):  # PE: matmuls, rhs = x fp8
        ht, base, njc = SUPERS[k]
        po = psum_out[ht]
        wc = wc_t[k]
        for j in range(njc):
            c = base + j
            nc.tensor.matmul(
                po[:], wc[:, j, :], x_slice(c),
                start=(c == 0), stop=(c == N_CHUNK - 1),
                skip_group_check=True,
            )
        wc_t[k] = None

    # remaining bulk DMAs are interleaved into the queues as the pipeline
    # advances, roughly in consumption order ("sx" = x piece via SP)
    dma_at = {0: [("x", 0)], 1: [("x", 1)], 2: [("wt", 1)],
              3: [("x", 2), ("wt", 2)], 4: [("wt", 3)],
              5: [("sx", 3)], 6: [("sx", 4)]}
    # PE padding after early stages: absorbs dependency gaps so the tensor
    # engine never idles (an idle gap resets its clock to 1.2 GHz for 3us)
    pe_pad = {0: 8, 1: 6, 2: 4, 3: 2}

    for k in range(n_sup + 4):
        if k < n_sup:
            sV(k)
        # DVE step: every op's inputs were produced >= 1 full step ago (the
        # evacuation sG lags its transpose by a whole step), so the in-order
        # DVE queue never stalls mid-step and paces at pure throughput.
        if 0 <= k - 2 < n_sup:
            sF(k - 2)
            pad(pe_pad.get(k - 2, 0))
        if 0 <= k - 3 < n_sup:
            sG(k - 3)
        if 0 <= k - 1 < n_sup:
            sQ2(k - 1)
            sQ3(k - 1)
            sP(k - 1)
        if k < n_sup:
            sQ1(k)
        for kind, g in dma_at.get(k, []):
            if kind == "x":
                dma_x(g)
            elif kind == "sx":
                dma_x(g, eng=nc.sync)
            else:
                dma_wt(g)
        if 0 <= k - 4 < n_sup:
            sH(k - 4)

    for ht in range(N_HT):
        res = res_pool.tile([128, B], fp8, tag="res", name=f"{R}res{ht}")
        nc.vector.tensor_scalar(
            res[:], psum_out[ht][:], thr_t[ht][:], None, op0=ALU.is_ge
        )
        # separate queues so the two output transfers overlap
        eng = nc.sync if ht == 0 else nc.scalar
        eng.dma_start(out[ht * 128:(ht + 1) * 128, :], res[:])


def _get_nc(reps=1):
    key = f"nc{reps}"
    if key not in _CACHED:
        _CACHED[key] = _build_bass(reps)
    return _CACHED[key]


def kernel(**inputs):
    global LAST_RESULTS
    from concourse.bass_utils import run_bass_kernel_spmd

    x = np.asarray(inputs["x"], dtype=np.float32)
    W = np.asarray(inputs["W"], dtype=np.float32)
    b = np.asarray(inputs["b"], dtype=np.float32)
    tau_m = np.asarray(inputs["tau_m"], dtype=np.float32)
    tau_n = np.asarray(inputs["tau_n"], dtype=np.float32)
    mask = np.asarray(inputs["mask"], dtype=np.float32)

    fp16 = np.float16
    fp8 = ml_dtypes.float8_e4m3
    # contraction layout i = p*32 + c (pure reshape of the transposed x / W)
    xr = np.ascontiguousarray(x.T).reshape(128, N_CHUNK, B).astype(fp8)
    # centered branch index t' = idx - 1.5, columns permuted to (c*128 + p)
    idx = (mask[:, :, 1] + 2.0 * mask[:, :, 2] + 3.0 * mask[:, :, 3]) - 1.5
    idx = np.ascontiguousarray(
        idx.reshape(H, 128, N_CHUNK).swapaxes(1, 2).reshape(H, I)
    ).astype(fp16)
    W16 = W.astype(fp16)

    nc = _get_nc()
    in_maps = []
    for c in range(NCORES):
        hs = slice(c * H_LOC, (c + 1) * H_LOC)
        par6 = np.concatenate(
            [tau_n[hs], tau_m[hs, None], b[hs, None]], axis=1
        ).astype(np.float32)                       # [256, 6]
        par = np.concatenate([par6[0:128], par6[128:256]], axis=1)  # [128, 12]
        in_maps.append({
            "x": xr,
            "wT": np.ascontiguousarray(W16[hs].T.reshape(128, N_CHUNK, H_LOC)),
            "idx": np.ascontiguousarray(idx[hs]),
            "par": np.ascontiguousarray(par),
        })

    try:
        res = run_bass_kernel_spmd(
            nc, in_maps, core_ids=list(range(NCORES)), trace=TRACE,
        )
    except Exception:
        if not TRACE:
            raise
        # tracing needs the NTFF profiling hook, which not every
        # environment provides — rerun without it
        res = run_bass_kernel_spmd(
            nc, in_maps, core_ids=list(range(NCORES)), trace=False,
        )
    LAST_RESULTS = res
    outT = np.concatenate(
        [np.asarray(r["out"], dtype=np.float32) for r in res.results], axis=0
    )                                                                 # [H, B]
    return np.ascontiguousarray(outT.T)                               # [B, H]


# revision 33
# speedup vs baseline: 1.0345x; 1.0336x over previous
"""Trainium2 Bass kernel for the DH-LIF node single-step forward.

Math: the mask is one-hot over the branch dim NB, so

    spike = ( (1-beta) * (x @ (W + M).T + b) >= 1 )
    M[h,i] = oma[h, idx[h,i]],   oma[h,k] = 0.5*(1 - sigmoid(tau_n[h,k]))

The host losslessly re-encodes the mask as the centered branch index
t' = idx - 1.5 (fp16, values +-0.5/+-1.5 exact).  The device rebuilds M via
the even/odd interpolation basis

    v = t'^2 (= 0.25 or 2.25, exact in fp16)
    M = (e0 + e1*v) + t'*(o0 + o1*v)

whose intermediates all stay below ~0.08, so per-step fp16 rounding is
harmless (45 flipped spikes of 239 allowed, deterministic inputs).  M is
transposed on TensorE (fp16 identity, 1 cyc/row), the +W^T rides the
PSUM->SBUF evacuation (W^T ships pre-transposed fp16), and one fp16 x fp8
matmul per 128-chunk accumulates into f32 PSUM (x ships as fp8; spikes are
0/1, exact).  Threshold compares against 1/(1-beta) - b; the 0/1 result is
written back as fp8.

Contraction index layout: i = p*32 + c (p = SBUF partition, c = k-chunk), a
pure host reshape that makes the x / W^T DMAs fully contiguous per
partition (128 descriptors of 4 KiB per quarter-tensor transfer).

Engine split per supertile: Act {v = Square(t'), q2 = o1*v + o0 via
Identity scale/bias, W^T DMAs}, Pool {q1 = e1*v + e0, x DMAs},
DVE {q3 = q2*t', P = q1 + q3, wc = psum + W^T, spike threshold},
PE {transposes + matmuls + clock-warming dummies}, SP {idx/param/out DMAs}.

Sharding: hidden dim split across 8 cores (h_loc = 256); x replicated.
"""

import numpy as np
import ml_dtypes

B, I, H, NB = 512, 4096, 2048, 4
NCORES = 8
H_LOC = H // NCORES          # 256
N_HT = H_LOC // 128          # 2 partition tiles of hidden per core
N_CHUNK = I // 128           # 32 matmul k-chunks

# (ht, base_chunk, n_chunks): small supers at the head (fast pipeline fill)
# and tail (short drain), 8-chunk supers in the middle.
SUPERS = [
    (0, 0, 4), (0, 4, 4), (0, 8, 8), (1, 0, 8), (0, 16, 8),
    (1, 8, 8), (1, 16, 8), (0, 24, 8), (1, 24, 4), (1, 28, 4),
]
# idx DMA pieces: (ht, col_start, col_end), ordered by consumption
IDX_PIECES = [
    (0, 0, 1024), (0, 1024, 4096), (1, 0, 2048), (1, 2048, 4096),
]
# x / W^T DMA pieces: chunk ranges; the x tail pieces feed only the last
# matmul groups
X_PIECES = [(0, 8), (8, 16), (16, 24), (24, 28), (28, 32)]
WT_PIECES = [(0, 16), (16, 32)]
NWARM = 38

TRACE = False
LAST_RESULTS = None
_CACHED = {}


def _build_bass(reps=1):
    import concourse.bacc as bacc
    import concourse.mybir as mybir
    from concourse.tile import TileContext
    from concourse.masks import make_identity

    f32 = mybir.dt.float32
    fp16 = mybir.dt.float16
    fp8 = mybir.dt.float8e4
    AF = mybir.ActivationFunctionType
    ALU = mybir.AluOpType

    nc = bacc.Bacc("TRN2", target_bir_lowering=False, debug=False)

    x_in = nc.dram_tensor("x", [128, N_CHUNK, B], fp8, kind="ExternalInput")
    wT_in = nc.dram_tensor("wT", [128, N_CHUNK, H_LOC], fp16, kind="ExternalInput")
    idx_in = nc.dram_tensor("idx", [H_LOC, I], fp16, kind="ExternalInput")
    par_in = nc.dram_tensor("par", [128, 2 * (NB + 2)], f32, kind="ExternalInput")
    out = nc.dram_tensor("out", [H_LOC, B], fp8, kind="ExternalOutput")

    with TileContext(nc) as tc:
        with (
            tc.tile_pool(name="const", bufs=1) as const_pool,
            tc.tile_pool(name="xp", bufs=3) as x_pool,
            tc.tile_pool(name="wt", bufs=4) as wt_pool,
            tc.tile_pool(name="ix", bufs=len(IDX_PIECES)) as idx_pool,
            tc.tile_pool(name="pl", bufs=3) as plane_pool,
            tc.tile_pool(name="wc", bufs=3) as wc_pool,
            tc.tile_pool(name="res", bufs=2) as res_pool,
            tc.tile_pool(name="pt", bufs=2, space="PSUM") as psum_t_pool,
            tc.tile_pool(name="po", bufs=2, space="PSUM") as psum_o_pool,
            tc.tile_pool(name="pw", bufs=1, space="PSUM") as psum_w_pool,
        ):
            ident = const_pool.tile([128, 128], fp16)
            make_identity(nc, ident)

            # Clock warmup: keep the PE continuously busy from t=0 so it is
            # at 2.4 GHz (needs >3us back-to-back) when real work arrives.
            warm = psum_w_pool.tile([128, 128], f32, name="warm")

            def pad(n):
                for _ in range(n):
                    nc.tensor.matmul(warm[:], ident[:], ident[:],
                                     start=True, stop=True,
                                     skip_group_check=True)

            pad(NWARM)
            for rep in range(reps):
                _emit_rep(nc, tc, rep, ident, pad,
                          const_pool, x_pool, wt_pool, idx_pool, plane_pool,
                          wc_pool, res_pool, psum_t_pool, psum_o_pool,
                          x_in, wT_in, idx_in, par_in, out,
                          f32, fp16, fp8, AF, ALU)

    nc.compile()
    return nc


def _emit_rep(nc, tc, rep, ident, pad,
              const_pool, x_pool, wt_pool, idx_pool, plane_pool,
              wc_pool, res_pool, psum_t_pool, psum_o_pool,
              x_in, wT_in, idx_in, par_in, out,
              f32, fp16, fp8, AF, ALU):
    R = f"r{rep}_"
    n_sup = len(SUPERS)

    # ---- DMA issue, spread over three queues ----
    # Each DMA instruction holds its queue until the transfer completes, so
    # the head-of-pipeline pieces go on separate queues: idx piece 0 leads
    # the SP queue, the (single, packed) param tile rides Pool's SWDGE.
    par_all = const_pool.tile([128, 2 * (NB + 2)], f32, tag=f"{R}par",
                              name=f"{R}par")
    nc.gpsimd.dma_start(par_all[:], par_in[:, :])

    def par_col(ht, c0, c1):
        off = ht * (NB + 2)
        return par_all[:, off + c0:off + c1]

    idx_sb = {}

    def idx_piece_tile(pi):
        ht, c0, c1 = IDX_PIECES[pi]
        t = idx_pool.tile([128, c1 - c0], fp16, tag=f"ix{c1-c0}",
                          name=f"{R}ix{pi}")
        nc.sync.dma_start(t[:], idx_in[ht * 128:(ht + 1) * 128, c0:c1])
        idx_sb[pi] = t

    for pi in range(len(IDX_PIECES)):
        idx_piece_tile(pi)

    def idx_slice(ht, c0, c1):
        """SBUF view of centered-idx columns [c0*128, c1*128) of h-tile ht."""
        lo, hi = c0 * 128, c1 * 128
        for pi, (h, p0, p1) in enumerate(IDX_PIECES):
            if h == ht and p0 <= lo and hi <= p1:
                return idx_sb[pi][:, lo - p0:hi - p0]
        raise AssertionError((ht, c0, c1))

    # Act queue: dummy sigmoid first — forces the single activation-table
    # load (Sigmoid/Square/Identity share a set) off the critical path.
    # The sigmoids also gate Act's first bulk DMA so the idx pieces win the
    # DMA-engine queue.
    scr = const_pool.tile([128, 1], f32, tag=f"{R}scr", name=f"{R}scr")
    nc.vector.memset(scr[:], 0.0)
    scr2 = const_pool.tile([128, 1], f32, tag=f"{R}scr2", name=f"{R}scr2")
    nc.scalar.activation(scr2[:], scr[:], AF.Sigmoid)

    sig_t = []
    for ht in range(N_HT):
        sig_n = const_pool.tile([128, NB], f32, tag=f"{R}sn{ht}", name=f"{R}sn{ht}")
        nc.scalar.activation(sig_n[:], par_col(ht, 0, NB), AF.Sigmoid)
        beta = const_pool.tile([128, 1], f32, tag=f"{R}be{ht}", name=f"{R}be{ht}")
        nc.scalar.activation(beta[:], par_col(ht, NB, NB + 1), AF.Sigmoid)
        sig_t.append((sig_n, beta))

    # W^T and most x pieces go through Pool's SWDGE: descriptor generation
    # costs ~1us of Pool engine time but does NOT hold the queue through the
    # transfer (HWDGE queues do), so compute streams are never blocked.
    wt_t = {}

    def dma_wt(g, eng=None):
        c0, c1 = WT_PIECES[g]
        t = wt_pool.tile([128, c1 - c0, H_LOC], fp16, tag=f"wt{c1-c0}",
                         name=f"{R}wt{g}")
        (eng or nc.gpsimd).dma_start(t[:], wT_in[:, c0:c1, :])
        wt_t[g] = t

    def wt_slice(ht, base, njc):
        hsl = slice(ht * 128, (ht + 1) * 128)
        for g, (c0, c1) in enumerate(WT_PIECES):
            if c0 <= base and base + njc <= c1:
                return wt_t[g][:, base - c0:base - c0 + njc, hsl]
        raise AssertionError((base, njc))

    x_t = {}

    def dma_x(g, eng=None):
        c0, c1 = X_PIECES[g]
        t = x_pool.tile([128, c1 - c0, B], fp8, tag=f"x{c1-c0}", name=f"{R}x{g}")
        (eng or nc.gpsimd).dma_start(t[:], x_in[:, c0:c1, :])
        x_t[g] = t

    def x_slice(c):
        for g, (c0, c1) in enumerate(X_PIECES):
            if c0 <= c < c1:
                return x_t[g][:, c - c0, :]
        raise AssertionError(c)

    dma_x(0)
    dma_wt(0)

    # ---- coefficients (f32, tiny): even/odd basis over v = t'^2 ----
    #   E(v) = e0 + e1 v   through ((o1+o2)/2 @v=.25, (o0+o3)/2 @v=2.25)
    #   O(v) = o0_ + o1_ v through (o2-o1 @v=.25, (o3-o0)/3 @v=2.25)
    coef = []   # (e0, e1, oo0, oo1) APs per ht
    thr_t = []
    for ht in range(N_HT):
        sig_n, beta = sig_t[ht]
        oma = const_pool.tile([128, NB], f32, tag=f"{R}oma{ht}", name=f"{R}oma{ht}")
        # 0.5 * (1 - sigmoid(tau_n)) — includes the 0.5 dendritic scale
        nc.vector.tensor_scalar(oma[:], sig_n[:], -0.5, 0.5, op0=ALU.mult, op1=ALU.add)

        sc = const_pool.tile([128, 8], f32, tag=f"{R}sc{ht}", name=f"{R}sc{ht}")
        s_in, s_out = sc[:, 0:1], sc[:, 1:2]
        e0, e1, oo0, oo1 = sc[:, 2:3], sc[:, 3:4], sc[:, 4:5], sc[:, 5:6]
        O_in, O_out = sc[:, 6:7], sc[:, 7:8]
        nc.vector.tensor_tensor(s_in, oma[:, 1:2], oma[:, 2:3], ALU.add)
        nc.vector.tensor_tensor(s_out, oma[:, 0:1], oma[:, 3:4], ALU.add)
        # e1 = (s_out - s_in)/4 ; e0 = s_in/2 - e1/4
        nc.vector.tensor_tensor(e1, s_out, s_in, ALU.subtract)
        nc.vector.tensor_scalar(e1, e1, 0.25, None, op0=ALU.mult)
        nc.vector.tensor_scalar(t_ := sc[:, 0:1], s_in, 0.5, None, op0=ALU.mult)
        nc.vector.tensor_scalar(s_out, e1, -0.25, None, op0=ALU.mult)
        nc.vector.tensor_tensor(e0, t_, s_out, ALU.add)
        nc.vector.tensor_tensor(O_in, oma[:, 2:3], oma[:, 1:2], ALU.subtract)
        nc.vector.tensor_tensor(O_out, oma[:, 3:4], oma[:, 0:1], ALU.subtract)
        nc.vector.tensor_scalar(O_out, O_out, 1.0 / 3.0, None, op0=ALU.mult)
        # oo1 = (O_out - O_in)/2 ; oo0 = O_in - oo1/4
        nc.vector.tensor_tensor(oo1, O_out, O_in, ALU.subtract)
        nc.vector.tensor_scalar(oo1, oo1, 0.5, None, op0=ALU.mult)
        nc.vector.tensor_scalar(O_out, oo1, -0.25, None, op0=ALU.mult)
        nc.vector.tensor_tensor(oo0, O_in, O_out, ALU.add)
        coef.append((e0, e1, oo0, oo1))

        omb = const_pool.tile([128, 1], f32, tag=f"{R}ob{ht}", name=f"{R}ob{ht}")
        nc.vector.tensor_scalar(omb[:], beta[:], -1.0, 1.0, op0=ALU.mult, op1=ALU.add)
        rb = const_pool.tile([128, 1], f32, tag=f"{R}rb{ht}", name=f"{R}rb{ht}")
        nc.vector.reciprocal(rb[:], omb[:])
        thr = const_pool.tile([128, 1], f32, tag=f"{R}th{ht}", name=f"{R}th{ht}")
        nc.vector.tensor_tensor(thr[:], rb[:], par_col(ht, NB + 1, NB + 2),
                                ALU.subtract)
        thr_t.append(thr)

    # ---- main pipeline ----
    psum_out = [None] * N_HT
    for ht in range(N_HT):
        psum_out[ht] = psum_o_pool.tile([128, B], f32, tag="po", name=f"{R}po{ht}")

    v_t = [None] * n_sup
    q1_t = [None] * n_sup
    q2_t = [None] * n_sup
    q3_t = [None] * n_sup
    pt_t = [None] * n_sup
    wc_t = [None] * n_sup

    def tp(pool, k, nm):
        ht, base, njc = SUPERS[k]
        return pool.tile([128, njc * 128], fp16, tag=f"{nm}{njc}",
                         name=f"{R}{nm}{k}")

    def sV(k):  # Act: v = (t')^2
        ht, base, njc = SUPERS[k]
        v = tp(plane_pool, k, "v")
        v_t[k] = v
        nc.scalar.activation(v[:], idx_slice(ht, base, base + njc), AF.Square)

    def sQ2(k):  # Act: q2 = o1*v + o0  (Identity with per-partition scale/bias)
        ht, base, njc = SUPERS[k]
        _, _, oo0, oo1 = coef[ht]
        q2 = tp(plane_pool, k, "q2")
        q2_t[k] = q2
        nc.scalar.activation(q2[:], v_t[k][:], AF.Identity, bias=oo0, scale=oo1)

    def sQ1(k):  # Pool: q1 = e1*v + e0
        ht, base, njc = SUPERS[k]
        e0, e1, _, _ = coef[ht]
        q1 = tp(plane_pool, k, "q1")
        q1_t[k] = q1
        nc.gpsimd.tensor_scalar(q1[:], v_t[k][:], e1, e0, op0=ALU.mult, op1=ALU.add)

    def sQ3(k):  # DVE: q3 = q2 * t'
        ht, base, njc = SUPERS[k]
        q3 = tp(plane_pool, k, "q3")
        q3_t[k] = q3
        nc.vector.tensor_tensor(q3[:], q2_t[k][:], idx_slice(ht, base, base + njc),
                                ALU.mult)
        v_t[k] = None

    def sP(k):  # DVE: P = q1 + q3  (final M plane, in-place on q3)
        nc.vector.tensor_tensor(q3_t[k][:], q1_t[k][:], q3_t[k][:], ALU.add)
        q1_t[k] = None
        q2_t[k] = None

    def sF(k):  # PE: transposes into fp16 PSUM
        ht, base, njc = SUPERS[k]
        pt = psum_t_pool.tile([128, njc, 128], fp16, tag=f"pt{njc}",
                              name=f"{R}pt{k}")
        pt_t[k] = pt
        P = q3_t[k]
        for j in range(njc):
            nc.tensor.transpose(pt[:, j, :], P[:, j * 128:(j + 1) * 128], ident[:])

    def sG(k):  # DVE: wc = pt + W^T
        ht, base, njc = SUPERS[k]
        wc = wc_pool.tile([128, njc, 128], fp16, tag=f"wc{njc}", name=f"{R}wc{k}")
        wc_t[k] = wc
        nc.vector.tensor_tensor(wc[:], pt_t[k][:], wt_slice(ht, base, njc),
                                ALU.add)
        q3_t[k] = None
        pt_t[k] = None

    def sH(k):  # PE: matmuls, rhs = x fp8
        ht, base, njc = SUPERS[k]
        po = psum_out[ht]
        wc = wc_t[k]
        for j in range(njc):
            c = base + j
            nc.tensor.matmul(
                po[:], wc[:, j, :], x_slice(c),
                start=(c == 0), stop=(c == N_CHUNK - 1),
                skip_group_check=True,
            )
        wc_t[k] = None

    # remaining bulk DMAs are interleaved into the queues as the pipeline
    # advances, roughly in consumption order ("sx" = x piece via SP HWDGE,
    # plain "x"/"wt" = Pool SWDGE)
    dma_at = {1: [("x", 1)], 2: [("x", 2)], 3: [("wt", 1)],
              4: [("sx", 3)], 5: [("sx", 4)]}
    # PE padding after early transposes: absorbs dependency gaps so the
    # tensor engine never idles (an idle gap resets its clock to 1.2 GHz)
    pe_pad = {0: 8, 1: 6, 2: 4, 3: 2, 4: 2, 5: 2}

    # Stage lags: every consumer runs a full step behind its producer, so no
    # in-order engine queue ever stalls mid-step on a same-step result:
    #   Act: q2(k-1), v(k) | Pool: q1(k-1) | DVE: sG(k-4), q3(k-2), P(k-2)
    #   PE: tr(k-3), mm(k-5)
    for k in range(n_sup + 5):
        if 0 <= k - 1 < n_sup:
            sQ2(k - 1)
        if k < n_sup:
            sV(k)
        if 0 <= k - 4 < n_sup:
            sG(k - 4)
        if 0 <= k - 2 < n_sup:
            sQ3(k - 2)
            sP(k - 2)
        if 0 <= k - 3 < n_sup:
            sF(k - 3)
            pad(pe_pad.get(k - 3, 0))
        if 0 <= k - 1 < n_sup:
            sQ1(k - 1)
        for kind, g in dma_at.get(k, []):
            if kind == "x":
                dma_x(g)
            elif kind == "sx":
                dma_x(g, eng=nc.sync)
            else:
                dma_wt(g)
        if 0 <= k - 5 < n_sup:
            sH(k - 5)

    for ht in range(N_HT):
        res = res_pool.tile([128, B], fp8, tag="res", name=f"{R}res{ht}")
        nc.vector.tensor_scalar(
            res[:], psum_out[ht][:], thr_t[ht][:], None, op0=ALU.is_ge
        )
        # separate queues so the two output transfers overlap
        eng = nc.sync if ht == 0 else nc.scalar
        eng.dma_start(out[ht * 128:(ht + 1) * 128, :], res[:])


def _get_nc(reps=1):
    key = f"nc{reps}"
    if key not in _CACHED:
        _CACHED[key] = _build_bass(reps)
    return _CACHED[key]


def kernel(**inputs):
    global LAST_RESULTS
    from concourse.bass_utils import run_bass_kernel_spmd

    x = np.asarray(inputs["x"], dtype=np.float32)
    W = np.asarray(inputs["W"], dtype=np.float32)
    b = np.asarray(inputs["b"], dtype=np.float32)
    tau_m = np.asarray(inputs["tau_m"], dtype=np.float32)
    tau_n = np.asarray(inputs["tau_n"], dtype=np.float32)
    mask = np.asarray(inputs["mask"], dtype=np.float32)

    fp16 = np.float16
    fp8 = ml_dtypes.float8_e4m3
    # contraction layout i = p*32 + c (pure reshape of the transposed x / W)
    xr = np.ascontiguousarray(x.T).reshape(128, N_CHUNK, B).astype(fp8)
    # centered branch index t' = idx - 1.5, columns permuted to (c*128 + p)
    idx = (mask[:, :, 1] + 2.0 * mask[:, :, 2] + 3.0 * mask[:, :, 3]) - 1.5
    idx = np.ascontiguousarray(
        idx.reshape(H, 128, N_CHUNK).swapaxes(1, 2).reshape(H, I)
    ).astype(fp16)
    W16 = W.astype(fp16)

    nc = _get_nc()
    in_maps = []
    for c in range(NCORES):
        hs = slice(c * H_LOC, (c + 1) * H_LOC)
        par6 = np.concatenate(
            [tau_n[hs], tau_m[hs, None], b[hs, None]], axis=1
        ).astype(np.float32)                       # [256, 6]
        par = np.concatenate([par6[0:128], par6[128:256]], axis=1)  # [128, 12]
        in_maps.append({
            "x": xr,
            "wT": np.ascontiguousarray(W16[hs].T.reshape(128, N_CHUNK, H_LOC)),
            "idx": np.ascontiguousarray(idx[hs]),
            "par": np.ascontiguousarray(par),
        })

    try:
        res = run_bass_kernel_spmd(
            nc, in_maps, core_ids=list(range(NCORES)), trace=TRACE,
        )
    except Exception:
        if not TRACE:
            raise
        # tracing needs the NTFF profiling hook, which not every
        # environment provides — rerun without it
        res = run_bass_kernel_spmd(
            nc, in_maps, core_ids=list(range(NCORES)), trace=False,
        )
    LAST_RESULTS = res
    outT = np.concatenate(
        [np.asarray(r["out"], dtype=np.float32) for r in res.results], axis=0
    )                                                                 # [H, B]
    return np.ascontiguousarray(outT.T)                               # [B, H]
